# revision 1
# baseline (speedup 1.0000x reference)
"""Trainium2 Bass kernel for the DLI loss (ragged segment means -> pairwise NLL).

Math reduction: see _host_finish. Heavy work = ragged segment SUM of
encoder_output as a masked matmul seg[T,D] = M[S,T]^T @ x[S,D], data-parallel
over 8 cores (4 batches each).

Pipeline notes (from trace analysis):
- HBM-DMA bound: 4 MB tiles (32 KB/partition descriptors) stream at ~410 GB/s;
  smaller descriptors degrade sharply (1 MB -> 341 GB/s). Batches 0-2 use two
  4 MB tiles; batch 3 tapers 16/8/4/4 chunks so the post-stream backlog is
  short. The taper tiles get dedicated SBUF slots so their DMA triggers never
  wait on casts.
- The x-stream triggers (Sync engine) must never sit behind a trigger with a
  late-satisfied wait, and the cast engines must never wait on late work:
  masks are computed upfront, the dots run AFTER all casts at the end of the
  DVE program (all four psum banks stay live), and batch 3's raw segment sum
  is evacuated by ACT and shipped to the host (host applies wl/wr), so no
  serial mask->matmul->dots chain couples consecutive batches.
- xb (bf16 cast output) slots are per-piece with bufs=8 (~2 tiles of slack)
  so transient matmul lag cannot back-pressure the casts that pace the
  stream triggers.
- ends/wlr broadcasts ride the ACT HWDGE ring (nc.scalar.dma_start): their
  tiny descriptors would otherwise stall the x FIFO (~6 us for 512 tiny
  descriptors observed).

bf16 matmul operands (mask is exact 0/1); the loss averages 64512 pairs so
bf16 noise washes out to ~3e-7 relative error (measured).
"""

import sys
import os

sys.path.insert(0, "/opt/trn_rl_repo")

_jp = os.environ.get("JAX_PLATFORMS")
if _jp is not None and "axon" not in _jp and "jax" not in sys.modules:
    del os.environ["JAX_PLATFORMS"]

import numpy as np

B, S, D, T = 32, 4096, 512, 64
N_CORES = 8
BPC = B // N_CORES          # batches per core
P = 128                     # SBUF partitions
NCH = S // P                # 32 chunks of [128, D] per batch
CPW = 4                     # chunks per cast piece (pieces alternate ACT/DVE)
RPP = 16                    # max chunks per tile (4 MB)

_PROGRAM_CACHE = {}

# (start_row, chunks, chunk_offset); s = row0 + ch*p + c at [p, c].
STD_TILES = [(0, 16, 0), (2048, 16, 16)]
LAST_TILES = [(0, 16, 0), (2048, 8, 16), (3072, 4, 24), (3584, 4, 28)]


def _build_program():
    from contextlib import ExitStack

    import concourse.bacc as bacc
    import concourse.mybir as mybir
    import concourse.tile as tile

    f32 = mybir.dt.float32
    bf16 = mybir.dt.bfloat16

    nc = bacc.Bacc(
        "TRN2", target_bir_lowering=False, debug=False, enable_asserts=False
    )

    x_d = nc.dram_tensor("x", [BPC, S, D], f32, kind="ExternalInput").ap()
    ends_d = nc.dram_tensor("endsb", [BPC, T], f32, kind="ExternalInput").ap()
    wlr_d = nc.dram_tensor("wlr", [2, D], f32, kind="ExternalInput").ap()
    out_d = nc.dram_tensor("out", [T, BPC - 1, 2], f32, kind="ExternalOutput").ap()
    seg3_d = nc.dram_tensor("seg3", [T, D], f32, kind="ExternalOutput").ap()

    tilings = [STD_TILES] * (BPC - 1) + [LAST_TILES]

    with tile.TileContext(nc) as tc, ExitStack() as ctx:
        singles = ctx.enter_context(tc.tile_pool(name="singles", bufs=1))
        xpool = ctx.enter_context(tc.tile_pool(name="xp", bufs=3))
        bpool = ctx.enter_context(tc.tile_pool(name="bp", bufs=10))
        mpool = ctx.enter_context(tc.tile_pool(name="mp", bufs=1))
        spool = ctx.enter_context(tc.tile_pool(name="sp", bufs=1))
        ppool = ctx.enter_context(tc.tile_pool(name="pp", bufs=1, space="PSUM"))

        dma_list = [(b, t) for b in range(BPC) for t in range(len(tilings[b]))]

        def x_dma(b, t):
            row0, ch, _ = tilings[b][t]
            if b == BPC - 1 and t >= 2:
                # Dedicated slots: taper triggers must not wait on casts.
                xt = xpool.tile([P, ch, D], f32, tag=f"tp{t}", bufs=1)
                nc.sync.dma_start(
                    xt[:],
                    x_d[b][row0 : row0 + ch * P, :].rearrange(
                        "(p c) d -> p c d", c=ch
                    ),
                )
                return xt
            xt = xpool.tile([P, RPP, D], f32, tag="xt")
            nc.sync.dma_start(
                xt[:, :ch, :],
                x_d[b][row0 : row0 + ch * P, :].rearrange("(p c) d -> p c d", c=ch),
            )
            return xt

        # First x tile before any setup work.
        xt_next = x_dma(0, 0)

        # Position index tables (gpsimd). The last batch's first tile
        # matches STD_TILES, so iota3 only stores the taper chunks (16..31).
        iota_t = singles.tile([P, NCH, T], f32, tag="iota_t")
        iota3 = singles.tile([P, NCH - 16, T], f32, tag="iota3")
        for row0, ch, coff in STD_TILES:
            nc.gpsimd.iota(
                iota_t[:, coff : coff + ch, :],
                [[1, ch], [0, T]],
                base=row0,
                channel_multiplier=ch,
                allow_small_or_imprecise_dtypes=True,
            )
        for row0, ch, coff in LAST_TILES[1:]:
            nc.gpsimd.iota(
                iota3[:, coff - 16 : coff - 16 + ch, :],
                [[1, ch], [0, T]],
                base=row0,
                channel_multiplier=ch,
                allow_small_or_imprecise_dtypes=True,
            )

        # ends (one broadcast) + wlr on the ACT HWDGE ring: keeps their tiny
        # descriptors out of the x-stream FIFO; the triggers sit at the top
        # of the ACT program with no wait conditions.
        ends_t = singles.tile([P, BPC, T], f32)
        nc.scalar.dma_start(
            ends_t[:], ends_d.unsqueeze(0).to_broadcast((P, BPC, T))
        )
        wlr_t = singles.tile([T, 2, D], f32)
        nc.scalar.dma_start(wlr_t[:], wlr_d.unsqueeze(0).to_broadcast((T, 2, D)))

        out_t = singles.tile([T, BPC - 1, 2], f32)
        seg3_t = singles.tile([T, D], f32)

        # mask[p,i,t] = (s <= end_t) - (s <= end_{t-1}) in {0,1}, bf16.
        # mask(b0) is emitted upfront; the rest are interleaved after the
        # first tiles' casts so they never monopolize DVE (4 back-to-back
        # masks would delay the casts that pace the xt-slot recycle and
        # thus the stream triggers).
        def emit_mask(b):
            cmpe = mpool.tile([P, NCH, T], bf16, tag="cmpe")
            mask = mpool.tile([P, NCH, T], bf16, tag=f"mask{b}")
            if b == BPC - 1:
                nc.vector.tensor_tensor(
                    cmpe[:, :16, :],
                    iota_t[:, :16, :],
                    ends_t[:, b : b + 1, :].to_broadcast((P, 16, T)),
                    op=mybir.AluOpType.is_le,
                )
                nc.vector.tensor_tensor(
                    cmpe[:, 16:, :],
                    iota3[:],
                    ends_t[:, b : b + 1, :].to_broadcast((P, NCH - 16, T)),
                    op=mybir.AluOpType.is_le,
                )
            else:
                nc.vector.tensor_tensor(
                    cmpe[:],
                    iota_t[:],
                    ends_t[:, b : b + 1, :].to_broadcast((P, NCH, T)),
                    op=mybir.AluOpType.is_le,
                )
            nc.vector.tensor_sub(
                mask[:, :, 1:], cmpe[:, :, 1:], cmpe[:, :, : T - 1]
            )
            nc.vector.tensor_copy(mask[:, :, 0:1], cmpe[:, :, 0:1])
            return mask

        masks = [emit_mask(0)]

        psums = []
        tile_counter = 0
        dma_iter = iter(dma_list[1:])
        n_pieces_total = sum(
            (ch + CPW - 1) // CPW for tl in tilings for _, ch, _ in tl
        )
        piece_idx = 0
        for b in range(BPC):
            mask = None
            psum = ppool.tile([T, D], f32, tag=f"ps{b}")
            psums.append(psum)
            for t, (row0, ch, coff) in enumerate(tilings[b]):
                xt = xt_next
                nxt = next(dma_iter, None)
                if nxt is not None:
                    xt_next = x_dma(*nxt)
                npieces = (ch + CPW - 1) // CPW
                mask = masks[b]
                for q in range(npieces):
                    sl = slice(q * CPW, min((q + 1) * CPW, ch))
                    pw = sl.stop - sl.start
                    use_act = piece_idx % 2 == 0
                    if piece_idx == n_pieces_total - 1:
                        use_act = False       # last piece on the faster DVE
                    elif piece_idx == n_pieces_total - 2:
                        use_act = True        # ...in parallel with ACT
                    piece_idx += 1
                    xb = bpool.tile([P, CPW, D], bf16, tag="xb")
                    eng = nc.scalar.copy if use_act else nc.vector.tensor_copy
                    eng(xb[:, :pw, :], xt[:, sl, :])
                    for c in range(pw):
                        i = coff + sl.start + c
                        nc.tensor.matmul(
                            psum[:],
                            mask[:, i, :],
                            xb[:, c, :],
                            start=(i == 0),
                            stop=(i == NCH - 1),
                        )
                tile_counter += 1
                if tile_counter <= BPC - 1:
                    masks.append(emit_mask(tile_counter))

        # Dots for batches 0..2 at the END of the DVE program — they never
        # block casts. All psum banks stay live (4 of 8 banks used).
        for b in range(BPC - 1):
            for d_ in range(2):
                scratch = spool.tile([T, D], f32, tag=f"scr{d_}")
                nc.vector.tensor_mul(scratch[:], psums[b][:], wlr_t[:, d_, :])
                nc.vector.reduce_sum(
                    out_t[:, b, d_ : d_ + 1],
                    scratch[:],
                    axis=mybir.AxisListType.X,
                )
        # Batch 3: raw segment sums evacuated by ACT; host applies wl/wr.
        nc.scalar.copy(seg3_t[:], psums[BPC - 1][:])

        # Output triggers after every x trigger in Sync program order.
        nc.sync.dma_start(out_d[:], out_t[:])
        nc.sync.dma_start(seg3_d[:], seg3_t[:])

    nc.compile()
    return nc


def _host_prep(encoder_output, W, b, his_turn_end_ids):
    x = np.ascontiguousarray(np.asarray(encoder_output, dtype=np.float32))
    W = np.asarray(W, dtype=np.float32)
    bias = np.asarray(b, dtype=np.float32)
    ends = np.asarray(his_turn_end_ids).astype(np.int64)

    ends_prev = np.concatenate(
        [np.full((B, 1), -1, np.int64), ends[:, :-1]], axis=1
    )
    endsb = ends.astype(np.float32)  # [B, T]

    wlr = np.stack([W[:D, 1] - W[:D, 0], W[D:, 1] - W[D:, 0]], axis=0)  # [2, D]
    wlr = np.ascontiguousarray(wlr, dtype=np.float32)
    bd = np.float64(np.float32(bias[1]) - np.float32(bias[0]))

    counts = (ends - ends_prev).astype(np.float64)  # [B, T]
    return x, endsb, wlr, bd, counts


def _host_finish(A0, C0, counts, bd):
    A = A0.astype(np.float64) / counts
    C = C0.astype(np.float64) / counts
    u = A[:, :, None] + C[:, None, :] + bd  # [B, T, T]
    j = np.arange(T)[:, None]
    k = np.arange(T)[None, :]
    tri = k < j
    adj = k == (j - 1)
    nll = np.where(adj, np.logaddexp(0.0, -u), np.logaddexp(0.0, u))
    n_pairs = B * (T * (T - 1) // 2)
    loss = np.sum(np.where(tri, nll, 0.0)) / n_pairs
    return np.asarray(loss, dtype=np.float32)


def kernel(encoder_output, W, b, his_turn_end_ids):
    from concourse.bass_utils import run_bass_kernel_spmd

    x, endsb, wlr, bd, counts = _host_prep(encoder_output, W, b, his_turn_end_ids)

    if "nc" not in _PROGRAM_CACHE:
        _PROGRAM_CACHE["nc"] = _build_program()
    nc = _PROGRAM_CACHE["nc"]

    in_maps = [
        {
            "x": x[i * BPC : (i + 1) * BPC],
            "endsb": endsb[i * BPC : (i + 1) * BPC],
            "wlr": wlr,
        }
        for i in range(N_CORES)
    ]
    trace = bool(int(os.environ.get("BASS_KERNEL_TRACE", "0")))
    kw = {}
    if os.environ.get("BASS_KERNEL_TMPDIR"):
        kw["tmpdir"] = os.environ["BASS_KERNEL_TMPDIR"]
    res = run_bass_kernel_spmd(nc, in_maps, list(range(N_CORES)), trace=trace, **kw)
    _PROGRAM_CACHE["last_results"] = res

    A0 = np.empty((B, T), np.float64)
    C0 = np.empty((B, T), np.float64)
    for i, r in enumerate(res.results):
        dots = r["out"]  # [T, BPC-1, 2]
        seg3 = r["seg3"].astype(np.float64)  # [T, D]
        for j in range(BPC - 1):
            A0[i * BPC + j] = dots[:, j, 0]
            C0[i * BPC + j] = dots[:, j, 1]
        A0[i * BPC + BPC - 1] = seg3 @ wlr[0].astype(np.float64)
        C0[i * BPC + BPC - 1] = seg3 @ wlr[1].astype(np.float64)
    return _host_finish(A0, C0, counts, bd)



# revision 2
# speedup vs baseline: 1.8725x; 1.8725x over previous
"""Trainium2 Bass kernel for the DLI loss (ragged segment means -> pairwise NLL).

Math reduction: see _host_finish. Heavy work = ragged segment SUM of
encoder_output as a masked matmul seg[T,D] = M[S,T]^T @ x[S,D], data-parallel
over 8 cores (4 batches each).

Design (v2, fp8 streaming):
- x is quantized to fp8 e4m3 on the host and streamed as one 2 MB tile per
  batch ([128, 32, 512], 16 KB/partition descriptors). HBM traffic per core
  drops 4x vs fp32 (8 MB), which is the roofline for this memory-bound
  kernel. Loss rel-err from e4m3 quantization measured at 5.9e-5 (the pair
  average washes out per-element noise).
- All four x DMAs get dedicated SBUF slots and are queued back-to-back on
  the Sync ring at program start with no wait conditions.
- Segment masks (exact 0/1, fp8) are built on DVE from a single f32 iota
  table (row = p*32 + c for every batch tile) minus-compare against the
  per-batch ends, then fed as the stationary operand of fp8 DoubleRow
  matmuls (2 chunks per instruction, 0.5 cycles/row) accumulating into one
  PSUM bank per batch.
- ACT evacuates each batch's raw seg sums from PSUM and ships them on the
  ACT HWDGE ring (keeps the tiny descriptors off the x FIFO); the host
  applies wl/wr, the count division, and the closed-form pairwise NLL.
"""

import sys
import os

sys.path.insert(0, "/opt/trn_rl_repo")

_jp = os.environ.get("JAX_PLATFORMS")
if _jp is not None and "axon" not in _jp and "jax" not in sys.modules:
    del os.environ["JAX_PLATFORMS"]

import numpy as np
import ml_dtypes

B, S, D, T = 32, 4096, 512, 64
N_CORES = 8
BPC = B // N_CORES          # batches per core
P = 128                     # SBUF partitions
NCH = S // P                # 32 chunks of [128, D] per batch; row = p*NCH + c

_PROGRAM_CACHE = {}


def _build_program():
    from contextlib import ExitStack

    import concourse.bacc as bacc
    import concourse.mybir as mybir
    import concourse.tile as tile

    f32 = mybir.dt.float32
    fp8 = mybir.dt.float8e4

    nc = bacc.Bacc(
        "TRN2", target_bir_lowering=False, debug=False, enable_asserts=False
    )

    x_d = nc.dram_tensor("x", [BPC, S, D], fp8, kind="ExternalInput").ap()
    ends_d = nc.dram_tensor("endsb", [BPC, T], f32, kind="ExternalInput").ap()
    seg_d = nc.dram_tensor("seg", [BPC, T, D], f32, kind="ExternalOutput").ap()

    with tile.TileContext(nc) as tc, ExitStack() as ctx:
        singles = ctx.enter_context(tc.tile_pool(name="singles", bufs=1))
        xpool = ctx.enter_context(tc.tile_pool(name="xp", bufs=1))
        mpool = ctx.enter_context(tc.tile_pool(name="mp", bufs=2))
        ppool = ctx.enter_context(tc.tile_pool(name="pp", bufs=1, space="PSUM"))

        # x stream: one dedicated slot per batch, all triggers queued on the
        # Sync ring upfront with no wait conditions.
        xts = []
        for b in range(BPC):
            xt = xpool.tile([P, NCH, D], fp8, tag=f"xt{b}", bufs=1)
            nc.sync.dma_start(
                xt[:],
                x_d[b].rearrange("(p c) d -> p c d", c=NCH),
            )
            xts.append(xt)

        # ends broadcast on the ACT HWDGE ring (off the x FIFO).
        ends_t = singles.tile([P, BPC, T], f32)
        nc.scalar.dma_start(
            ends_t[:], ends_d.unsqueeze(0).to_broadcast((P, BPC, T))
        )

        # Position index table: value[p, c, t] = p*NCH + c, shared by all
        # batch tiles (identical layout).
        iota_t = singles.tile([P, NCH, T], f32, tag="iota_t")
        nc.gpsimd.iota(
            iota_t[:],
            [[1, NCH], [0, T]],
            base=0,
            channel_multiplier=NCH,
            allow_small_or_imprecise_dtypes=True,
        )

        # mask[p,c,t] = (s <= end_t) - (s <= end_{t-1}) in {0,1}, fp8.
        def emit_mask(b):
            cmpe = mpool.tile([P, NCH, T], fp8, tag="cmpe")
            mask = mpool.tile([P, NCH, T], fp8, tag=f"mask{b}", bufs=1)
            nc.vector.tensor_tensor(
                cmpe[:],
                iota_t[:],
                ends_t[:, b : b + 1, :].to_broadcast((P, NCH, T)),
                op=mybir.AluOpType.is_le,
            )
            nc.vector.tensor_sub(
                mask[:, :, 1:], cmpe[:, :, 1:], cmpe[:, :, : T - 1]
            )
            nc.vector.tensor_copy(mask[:, :, 0:1], cmpe[:, :, 0:1])
            return mask

        masks = [emit_mask(b) for b in range(BPC)]

        # fp8 DoubleRow matmuls: 2 chunks per instruction, 16 per batch.
        seg_ts = []
        for b in range(BPC):
            psum = ppool.tile([T, D], f32, tag=f"ps{b}")
            for i in range(NCH // 2):
                nc.tensor.matmul(
                    psum[:],
                    masks[b][:, 2 * i : 2 * i + 2, :],
                    xts[b][:, 2 * i : 2 * i + 2, :],
                    start=(i == 0),
                    stop=(i == NCH // 2 - 1),
                    perf_mode=mybir.MatmulPerfMode.DoubleRow,
                )
            # Evacuate raw seg sums on ACT; ship on the ACT ring so earlier
            # batches' outputs go out while the x stream is still running.
            seg_t = singles.tile([T, D], f32, tag=f"seg{b}")
            nc.scalar.copy(seg_t[:], psum[:])
            nc.scalar.dma_start(seg_d[b], seg_t[:])
            seg_ts.append(seg_t)

    nc.compile()
    return nc


def _host_prep(encoder_output, W, b, his_turn_end_ids):
    x = np.asarray(encoder_output, dtype=np.float32)
    xq = x.astype(ml_dtypes.float8_e4m3)
    W = np.asarray(W, dtype=np.float32)
    bias = np.asarray(b, dtype=np.float32)
    ends = np.asarray(his_turn_end_ids).astype(np.int64)

    ends_prev = np.concatenate(
        [np.full((B, 1), -1, np.int64), ends[:, :-1]], axis=1
    )
    endsb = ends.astype(np.float32)  # [B, T]

    wlr = np.stack([W[:D, 1] - W[:D, 0], W[D:, 1] - W[D:, 0]], axis=0)  # [2, D]
    bd = np.float64(np.float32(bias[1]) - np.float32(bias[0]))

    counts = (ends - ends_prev).astype(np.float64)  # [B, T]
    return xq, endsb, wlr, bd, counts


def _host_finish(A0, C0, counts, bd):
    A = A0 / counts
    C = C0 / counts
    u = A[:, :, None] + C[:, None, :] + bd  # [B, T, T]
    j = np.arange(T)[:, None]
    k = np.arange(T)[None, :]
    tri = k < j
    adj = k == (j - 1)
    nll = np.where(adj, np.logaddexp(0.0, -u), np.logaddexp(0.0, u))
    n_pairs = B * (T * (T - 1) // 2)
    loss = np.sum(np.where(tri, nll, 0.0)) / n_pairs
    return np.asarray(loss, dtype=np.float32)


def kernel(encoder_output, W, b, his_turn_end_ids):
    from concourse.bass_utils import run_bass_kernel_spmd

    xq, endsb, wlr, bd, counts = _host_prep(encoder_output, W, b, his_turn_end_ids)

    if "nc" not in _PROGRAM_CACHE:
        _PROGRAM_CACHE["nc"] = _build_program()
    nc = _PROGRAM_CACHE["nc"]

    in_maps = [
        {
            "x": xq[i * BPC : (i + 1) * BPC],
            "endsb": endsb[i * BPC : (i + 1) * BPC],
        }
        for i in range(N_CORES)
    ]
    trace = bool(int(os.environ.get("BASS_KERNEL_TRACE", "0")))
    kw = {}
    if os.environ.get("BASS_KERNEL_TMPDIR"):
        kw["tmpdir"] = os.environ["BASS_KERNEL_TMPDIR"]
    res = run_bass_kernel_spmd(nc, in_maps, list(range(N_CORES)), trace=trace, **kw)
    _PROGRAM_CACHE["last_results"] = res

    wlr64 = wlr.astype(np.float64)
    A0 = np.empty((B, T), np.float64)
    C0 = np.empty((B, T), np.float64)
    for i, r in enumerate(res.results):
        seg = r["seg"].astype(np.float64)  # [BPC, T, D]
        A0[i * BPC : (i + 1) * BPC] = seg @ wlr64[0]
        C0[i * BPC : (i + 1) * BPC] = seg @ wlr64[1]
    return _host_finish(A0, C0, counts, bd)


# revision 3
# speedup vs baseline: 2.2810x; 1.2182x over previous
"""Trainium2 Bass kernel for the DLI loss (ragged segment means -> pairwise NLL).

Math reduction: see _host_finish. Heavy work = ragged PREFIX sums of
encoder_output as a masked matmul P[T,D] = C[S,T]^T @ x[S,D] with
C[s,t] = (s <= end_t); the host takes adjacent differences to recover
segment sums. Data-parallel over 8 cores (4 batches each).

Design (v3, fp8 streaming at the HBM roofline):
- x is quantized to fp8 e4m3 on the host (loss rel-err 5.9e-5, measured)
  and streamed as one 2 MB tile per batch ([128, 32, 512] fp8, 16 KB/
  partition descriptors, ~384 GB/s aggregate over 16 DMA engines). Batch 3
  tapers 16/8/4/4 chunks so the PE can chase the stream and the tail stays
  short. All triggers are queued on the Sync ring upfront, no waits.
- Prefix masks (is_le result, exact 0/1 fp8) are the matmul stationary
  directly -- no subtract/copy pass. One DVE tensor_tensor per batch
  (fp8 output runs at half DVE rate; 4 ops ~11 us, hidden under the
  stream).
- ends are broadcast to all 128 partitions with a K=1 fp32 ones-matmul
  (a 128-descriptor broadcast DMA is descriptor-issue-bound at ~60 ns
  each = 7.7 us of queue stall, measured in v2).
- gpsimd iota writes only a [P, 32, 1] column per layout; the compare
  reads it with a stride-0 broadcast AP.
- fp8 DoubleRow matmuls (2 chunks per instruction, 259 ns steady-state)
  accumulate into one PSUM bank per batch; ACT evacuates P to SBUF and
  the per-batch output DMAs ride the gpsimd ring (own queue, idle).
- Host: Q = P @ wlr in f64, A0/C0 = adjacent diffs of Q, count division,
  closed-form pairwise NLL.
"""

import sys
import os

sys.path.insert(0, "/opt/trn_rl_repo")

_jp = os.environ.get("JAX_PLATFORMS")
if _jp is not None and "axon" not in _jp and "jax" not in sys.modules:
    del os.environ["JAX_PLATFORMS"]

import numpy as np
import ml_dtypes

B, S, D, T = 32, 4096, 512, 64
N_CORES = 8
BPC = B // N_CORES          # batches per core
P = 128                     # SBUF partitions
NCH = S // P                # 32 chunks of [128, D] per batch; row = p*NCH + c

# (start_row, chunks, chunk_offset); s = row0 + p*ch + c_local.
STD_TILES = [(0, NCH, 0)]
LAST_TILES = [(0, 16, 0), (2048, 8, 16), (3072, 4, 24), (3584, 4, 28)]

_PROGRAM_CACHE = {}


def _build_program():
    from contextlib import ExitStack

    import concourse.bacc as bacc
    import concourse.mybir as mybir
    import concourse.tile as tile

    f32 = mybir.dt.float32
    fp8 = mybir.dt.float8e4

    nc = bacc.Bacc(
        "TRN2", target_bir_lowering=False, debug=False, enable_asserts=False
    )

    x_d = nc.dram_tensor("x", [BPC, S, D], fp8, kind="ExternalInput").ap()
    ends_d = nc.dram_tensor("endsb", [1, BPC * T], f32, kind="ExternalInput").ap()
    pfx_d = nc.dram_tensor("pfx", [BPC, T, D], f32, kind="ExternalOutput").ap()

    tilings = [STD_TILES] * (BPC - 1) + [LAST_TILES]

    with tile.TileContext(nc) as tc, ExitStack() as ctx:
        singles = ctx.enter_context(tc.tile_pool(name="singles", bufs=1))
        xpool = ctx.enter_context(tc.tile_pool(name="xp", bufs=1))
        mpool = ctx.enter_context(tc.tile_pool(name="mp", bufs=1))
        ppool = ctx.enter_context(tc.tile_pool(name="pp", bufs=1, space="PSUM"))

        # x stream: dedicated slot per tile, all triggers queued on the
        # Sync ring upfront with no wait conditions.
        xts = []
        for b in range(BPC):
            for t, (row0, ch, coff) in enumerate(tilings[b]):
                xt = xpool.tile([P, ch, D], fp8, tag=f"xt{b}_{t}", bufs=1)
                nc.sync.dma_start(
                    xt[:],
                    x_d[b][row0 : row0 + ch * P, :].rearrange(
                        "(p c) d -> p c d", c=ch
                    ),
                )
                xts.append(xt)

        # ends: tiny 1-descriptor load to partition 0 (ACT ring), then
        # broadcast to all partitions with a K=1 fp32 ones-matmul.
        ends_row = singles.tile([1, BPC * T], f32)
        nc.scalar.dma_start(ends_row[:], ends_d)
        ones_row = singles.tile([1, P], f32)
        nc.vector.memset(ones_row[:], 1.0)
        psum_e = ppool.tile([P, BPC * T], f32, tag="pse")
        nc.tensor.matmul(psum_e[:], ones_row[:], ends_row[:], start=True, stop=True)
        ends_t = singles.tile([P, BPC, T], f32)
        nc.vector.tensor_copy(
            ends_t[:], psum_e[:].rearrange("p (b t) -> p b t", b=BPC)
        )

        # Position columns: value[p, c, 0] = row0 + p*ch + c_local for the
        # owning tile layout; batches 0..2 share one column, batch 3 has
        # its own (taper layouts).
        iota_t = singles.tile([P, NCH, 1], f32, tag="iota_t")
        nc.gpsimd.iota(
            iota_t[:],
            [[1, NCH], [0, 1]],
            base=0,
            channel_multiplier=NCH,
            allow_small_or_imprecise_dtypes=True,
        )
        iota3 = singles.tile([P, NCH, 1], f32, tag="iota3")
        for row0, ch, coff in LAST_TILES:
            nc.gpsimd.iota(
                iota3[:, coff : coff + ch, :],
                [[1, ch], [0, 1]],
                base=row0,
                channel_multiplier=ch,
                allow_small_or_imprecise_dtypes=True,
            )

        # Prefix masks: cmpe[p,c,t] = (s <= end_t), fp8 {0,1}. One DVE op
        # per batch; stride-0 broadcast reads on both operands.
        cmpes = []
        for b in range(BPC):
            col = iota3 if b == BPC - 1 else iota_t
            cmpe = mpool.tile([P, NCH, T], fp8, tag=f"cmpe{b}", bufs=1)
            nc.vector.tensor_tensor(
                cmpe[:],
                col[:].to_broadcast((P, NCH, T)),
                ends_t[:, b : b + 1, :].to_broadcast((P, NCH, T)),
                op=mybir.AluOpType.is_le,
            )
            cmpes.append(cmpe)

        # fp8 DoubleRow matmuls: 2 chunks per instruction. P evac on ACT,
        # per-batch output DMA on the gpsimd ring (own queue).
        for b in range(BPC):
            psum = ppool.tile([T, D], f32, tag=f"ps{b}")
            pair = 0
            xt_i = sum(len(tilings[bb]) for bb in range(b))
            for t, (row0, ch, coff) in enumerate(tilings[b]):
                xt = xts[xt_i + t]
                for j in range(ch // 2):
                    nc.tensor.matmul(
                        psum[:],
                        cmpes[b][:, coff + 2 * j : coff + 2 * j + 2, :],
                        xt[:, 2 * j : 2 * j + 2, :],
                        start=(pair == 0),
                        stop=(pair == NCH // 2 - 1),
                        perf_mode=mybir.MatmulPerfMode.DoubleRow,
                    )
                    pair += 1
            pfx_t = singles.tile([T, D], f32, tag=f"pfx{b}")
            nc.scalar.copy(pfx_t[:], psum[:])
            nc.gpsimd.dma_start(pfx_d[b], pfx_t[:])

    nc.compile()
    return nc


def _host_prep(encoder_output, W, b, his_turn_end_ids):
    x = np.asarray(encoder_output, dtype=np.float32)
    xq = x.astype(ml_dtypes.float8_e4m3)
    W = np.asarray(W, dtype=np.float32)
    bias = np.asarray(b, dtype=np.float32)
    ends = np.asarray(his_turn_end_ids).astype(np.int64)

    ends_prev = np.concatenate(
        [np.full((B, 1), -1, np.int64), ends[:, :-1]], axis=1
    )
    endsb = ends.astype(np.float32)  # [B, T]

    wlr = np.stack([W[:D, 1] - W[:D, 0], W[D:, 1] - W[D:, 0]], axis=0)  # [2, D]
    bd = np.float64(np.float32(bias[1]) - np.float32(bias[0]))

    counts = (ends - ends_prev).astype(np.float64)  # [B, T]
    return xq, endsb, wlr, bd, counts


def _host_finish(A0, C0, counts, bd):
    A = A0 / counts
    C = C0 / counts
    u = A[:, :, None] + C[:, None, :] + bd  # [B, T, T]
    j = np.arange(T)[:, None]
    k = np.arange(T)[None, :]
    tri = k < j
    adj = k == (j - 1)
    nll = np.where(adj, np.logaddexp(0.0, -u), np.logaddexp(0.0, u))
    n_pairs = B * (T * (T - 1) // 2)
    loss = np.sum(np.where(tri, nll, 0.0)) / n_pairs
    return np.asarray(loss, dtype=np.float32)


def kernel(encoder_output, W, b, his_turn_end_ids):
    from concourse.bass_utils import run_bass_kernel_spmd

    xq, endsb, wlr, bd, counts = _host_prep(encoder_output, W, b, his_turn_end_ids)

    if "nc" not in _PROGRAM_CACHE:
        _PROGRAM_CACHE["nc"] = _build_program()
    nc = _PROGRAM_CACHE["nc"]

    in_maps = [
        {
            "x": xq[i * BPC : (i + 1) * BPC],
            "endsb": endsb[i * BPC : (i + 1) * BPC].reshape(1, BPC * T),
        }
        for i in range(N_CORES)
    ]
    trace = bool(int(os.environ.get("BASS_KERNEL_TRACE", "0")))
    kw = {}
    if os.environ.get("BASS_KERNEL_TMPDIR"):
        kw["tmpdir"] = os.environ["BASS_KERNEL_TMPDIR"]
    res = run_bass_kernel_spmd(nc, in_maps, list(range(N_CORES)), trace=trace, **kw)
    _PROGRAM_CACHE["last_results"] = res

    wlr64 = wlr.astype(np.float64)
    A0 = np.empty((B, T), np.float64)
    C0 = np.empty((B, T), np.float64)
    for i, r in enumerate(res.results):
        pfx = r["pfx"].astype(np.float64)  # [BPC, T, D]
        Q = pfx @ wlr64.T                  # [BPC, T, 2] prefix dots
        seg_dots = np.diff(Q, axis=1, prepend=0.0)  # [BPC, T, 2]
        A0[i * BPC : (i + 1) * BPC] = seg_dots[:, :, 0]
        C0[i * BPC : (i + 1) * BPC] = seg_dots[:, :, 1]
    return _host_finish(A0, C0, counts, bd)
